# revision 23
# baseline (speedup 1.0000x reference)
"""Causal single-head self-attention on 8 TRN2 NeuronCores.

Sharding: 8 cores = 4 batches x 2 cores/batch. Within a batch the 8
512-query chunks are split zigzag (core A owns chunks {0,3,4,7}, core B
{1,2,5,6}) so causal work balances (18 units each). Each core projects
K/V for the whole batch from its own copy of x (recompute beats
cross-core K/V exchange: a pair AllGather was measured at ~8us per
collective plus ~29us cc-engine warmup, far more than the ~10us of PE
it saves), computes Q only for its owned chunks, then does block-causal
flash-style attention without the row-max pass (scores here are O(1) so
exp never overflows) and a fused out-projection.

SPMD trick: one program runs on all 8 cores, so per-core differences
live in the DATA only. x rows are fed in a per-core storage permutation
that puts owned query chunks at uniform offsets (storage chunks
0,2,4,6). With QSLOT[g] == 2g, the boundary region of slot g is always
storage chunks 2g (own, diagonal) and 2g+1 (partner): the 4 diagonal
k-blocks use 4 SHARED staircase masks (identical across slots and
cores, built once on gpsimd via affine_select), and the 4 partner
k-blocks are all-keep or all-drop - handled for free by a per-core
per-slot bias column on the Exp activation (exp(s*scale - 50) == 0).

x is passed D-major bf16 (cast on host); x chunk loads are batched 3D
DMAs issued FIRST and split across the gpsimd and sync queues so issue
overhead doesn't serialize startup.

Layouts (partition dim first):
  xT   [128, 8, 4096]  bf16   x^T per d-chunk
  K^T  [128, 4096]     bf16   H-major keys
  Q^T  [128, 2048]     bf16   H-major owned queries
  V    [128, 32, 256]  bf16   token-major V tiles (PE-transposed from the
                              H-major projection); col 128 = ones column
  scores_T [k=128, q=512] PSUM; P_T = exp(scale*s + bias) bf16
  O [q=128, 128+1] accumulates in PSUM over k-blocks with P_T subtiles as
  the stationary operand and [V|1] moving; col 128 = softmax denominator.
  The 1/denominator scale rides the O PSUM->SBUF copy, so O^T (PE
  transpose) is already normalized; out-projection results need only a
  cast copy (split between vector and scalar queues) before the bf16
  DMA out.
"""

import ml_dtypes
import numpy as np
from contextlib import ExitStack

import concourse.bass as bass
import concourse.tile as tile
from concourse import bacc, mybir
from concourse.bass_utils import run_bass_kernel_spmd
from concourse.masks import make_identity

S, B, D, H = 4096, 4, 1024, 128
P = 128
QC = 512                  # query chunk
NSLOT = 4                 # owned chunks per core
DC = D // P               # 8 d-chunks
TT = S // P               # 32 token tiles / k-blocks
NKT = S // QC             # 8 key 512-chunks
SCALE = float(H) ** -0.5
MASK_BIAS = -50.0         # exp(s*SCALE + MASK_BIAS) rounds to 0 in bf16

# storage-order permutation of the 8 query chunks, per role. Queries the
# core owns sit at storage chunks 0,2,4,6; the first 2(g+1) storage
# chunks cover every true key needed by owned chunk g (extras masked).
SIGMA = {0: [0, 1, 3, 2, 4, 5, 7, 6], 1: [1, 0, 2, 3, 5, 4, 6, 7]}
QSLOT = [0, 2, 4, 6]      # storage chunk positions of owned queries

F32 = mybir.dt.float32
BF16 = mybir.dt.bfloat16


def _build_kernel():
    nc = bacc.Bacc("TRN2", target_bir_lowering=False, debug=False, num_devices=8)

    xbT = nc.dram_tensor("xbT", [D, S], BF16, kind="ExternalInput")
    wqT = nc.dram_tensor("wqT", [P, DC, H], BF16, kind="ExternalInput")
    wkT = nc.dram_tensor("wkT", [P, DC, H], BF16, kind="ExternalInput")
    wvT = nc.dram_tensor("wvT", [P, DC, H], BF16, kind="ExternalInput")
    woT = nc.dram_tensor("woT", [H, D], BF16, kind="ExternalInput")
    meta = nc.dram_tensor("meta", [P, NSLOT], F32, kind="ExternalInput")
    out = nc.dram_tensor("out", [NSLOT * QC, D], BF16, kind="ExternalOutput")

    with ExitStack() as ctx:
        tc = ctx.enter_context(tile.TileContext(nc))
        _body(ctx, tc, xbT.ap(), wqT.ap(), wkT.ap(), wvT.ap(), woT.ap(),
              meta.ap(), out.ap())

    nc.compile()
    return nc


def _body(ctx, tc, xbT, wqT, wkT, wvT, woT, meta, out):
    nc = tc.nc

    consts = ctx.enter_context(tc.tile_pool(name="consts", bufs=1))
    bigbuf = ctx.enter_context(tc.tile_pool(name="bigbuf", bufs=1))
    ptpool = ctx.enter_context(tc.tile_pool(name="pt", bufs=8))
    otmp_pool = ctx.enter_context(tc.tile_pool(name="otmp", bufs=6))
    ypool = ctx.enter_context(tc.tile_pool(name="y", bufs=4))
    psA = ctx.enter_context(tc.tile_pool(name="psA", bufs=3, space="PSUM"))
    psTr = ctx.enter_context(tc.tile_pool(name="psTr", bufs=1, space="PSUM"))
    psO = ctx.enter_context(tc.tile_pool(name="psO", bufs=4, space="PSUM"))

    # x^T view [p, c, s] of the D-major HBM tensor for batched 3D loads
    xbT_r = xbT.rearrange("(c p) s -> p c s", p=P)

    xT = bigbuf.tile([P, DC, S], BF16)
    k_sb = bigbuf.tile([P, S], BF16)
    vT_sb = bigbuf.tile([P, S], BF16)
    q_sb = bigbuf.tile([P, NSLOT * QC], BF16)
    v_sb = bigbuf.tile([P, TT, 2 * P], BF16)  # V k-blocks + ones col (padded stride)
    o_t = bigbuf.tile([P, NSLOT * NSLOT, P], BF16)  # O^T [h, q-tile, q], normalized
    rec_sb = bigbuf.tile([P, NSLOT * NSLOT], F32)   # 1/rowsum per q-tile column

    # x chunk loads FIRST (they gate the first projection), split across
    # the gpsimd and sync DMA queues; weights on sync behind the first x.
    wk_sb = consts.tile([P, DC, H], BF16)
    wv_sb = consts.tile([P, DC, H], BF16)
    wq_sb = consts.tile([P, DC, H], BF16)
    woT_sb = consts.tile([P, D], BF16)
    # priority loads: wk + the first few c-blocks of x chunk 0 gate the
    # very first projection matmuls
    nc.sync.dma_start(wk_sb[:], wkT)
    nc.scalar.dma_start(xT[:, 0:2, bass.ts(0, QC)],
                        xbT_r[:, 0:2, bass.ts(0, QC)])
    nc.sync.dma_start(xT[:, 2:4, bass.ts(0, QC)],
                      xbT_r[:, 2:4, bass.ts(0, QC)])
    nc.scalar.dma_start(xT[:, 4:8, bass.ts(0, QC)],
                        xbT_r[:, 4:8, bass.ts(0, QC)])
    nc.sync.dma_start(xT[:, 0:4, bass.ts(1, QC)],
                      xbT_r[:, 0:4, bass.ts(1, QC)])
    nc.scalar.dma_start(xT[:, 4:8, bass.ts(1, QC)],
                        xbT_r[:, 4:8, bass.ts(1, QC)])
    nc.sync.dma_start(wv_sb[:], wvT)
    nc.sync.dma_start(wq_sb[:], wqT)
    nc.sync.dma_start(woT_sb[:], woT)

    meta_sb = consts.tile([P, NSLOT], F32)
    nc.scalar.dma_start(meta_sb[:], meta)
    # 4 shared diagonal staircase masks: mask_j[k, q'] = 1.0 iff q' >= k + 128*j
    mask_sb = consts.tile([P, 4, QC], BF16)
    for j in range(4):
        nc.gpsimd.memset(mask_sb[:, j, :], 1.0)
        nc.gpsimd.affine_select(
            out=mask_sb[:, j, :], in_=mask_sb[:, j, :],
            compare_op=mybir.AluOpType.is_ge, fill=0.0,
            base=-128 * j, pattern=[[1, QC]], channel_multiplier=-1)
    ident = consts.tile([P, P], BF16)
    make_identity(nc, ident[:])
    nc.vector.memset(v_sb[:, :, H], 1.0)  # ones column for rowsum trick

    def project(w_sb, dst, src_kt, dst_kt=None):
        ps = psA.tile([P, QC], F32)
        for c in range(DC):
            nc.tensor.matmul(ps[:], lhsT=w_sb[:, c, :],
                             rhs=xT[:, c, bass.ts(src_kt, QC)],
                             start=(c == 0), stop=(c == DC - 1))
        nc.vector.tensor_copy(dst[:, bass.ts(src_kt if dst_kt is None else dst_kt,
                                             QC)], ps[:])

    def attention_slot(g):
        nb = 8 * (g + 1)  # padded extent in k-blocks
        po = [psO.tile([P, H + 1], F32, name="po") for _ in range(NSLOT)]

        for bk in range(nb):
            ps = psA.tile([P, QC], F32)
            nc.tensor.matmul(ps[:], lhsT=k_sb[:, bass.ts(bk, P)],
                             rhs=q_sb[:, bass.ts(g, QC)], start=True, stop=True)
            pt = ptpool.tile([P, QC], BF16)
            r = bk - 8 * g
            if r >= 4:  # partner chunk: all-keep or all-drop via bias column
                nc.scalar.activation(pt[:], ps[:],
                                     mybir.ActivationFunctionType.Exp,
                                     bias=meta_sb[:, g : g + 1], scale=SCALE)
            elif 0 <= r < 4:  # diagonal: exp+staircase mask in 256-col halves
                for hf in range(2):  # so PV(sub 0/1) starts during half 1
                    cols = bass.ds(hf * 2 * P, 2 * P)
                    nc.scalar.activation(pt[:, cols], ps[:, cols],
                                         mybir.ActivationFunctionType.Exp,
                                         scale=SCALE)
                    nc.vector.tensor_mul(pt[:, cols], pt[:, cols],
                                         mask_sb[:, r, cols])
            else:
                nc.scalar.activation(pt[:], ps[:],
                                     mybir.ActivationFunctionType.Exp,
                                     scale=SCALE)
            for sub in range(NSLOT):
                nc.tensor.matmul(po[sub][:], lhsT=pt[:, bass.ts(sub, P)],
                                 rhs=v_sb[:, bk, 0 : H + 1],
                                 start=(bk == 0), stop=(bk == nb - 1))
        return po

    def finish_slot(g, po):
        for sub in range(NSLOT):
            idx = g * NSLOT + sub
            nc.vector.reciprocal(rec_sb[:, idx : idx + 1], po[sub][:, H : H + 1])
            ob = otmp_pool.tile([P, P], BF16, name="ob")
            nc.vector.tensor_scalar_mul(ob[:], po[sub][:, 0:H],
                                        rec_sb[:, idx : idx + 1])
            pstr = psTr.tile([P, P], BF16, name="tr")
            nc.tensor.transpose(pstr[:], ob[:], ident[:])
            nc.vector.tensor_copy(o_t[:, idx, :], pstr[:])
            # out-projection for this q-tile; copies split vector/scalar
            y = ypool.tile([P, D], BF16)
            for half in range(2):
                ps = psA.tile([P, QC], F32)
                nc.tensor.matmul(ps[:], lhsT=o_t[:, idx, :],
                                 rhs=woT_sb[:, bass.ts(half, QC)],
                                 start=True, stop=True)
                nc.scalar.activation(y[:, bass.ts(half, QC)], ps[:],
                                     mybir.ActivationFunctionType.Copy)
            nc.sync.dma_start(out[bass.ts(idx, P), :], y[:])

    # Pipelined emission over key 512-chunks: project K/V (+V re-transpose
    # to token-major), project Q when its chunk lands, then run each slot's
    # attention + out-projection as soon as its extent is covered.
    pending = None
    for kt in range(NKT):
        if kt == 2:  # chunks 2-3 in two batched issues
            nc.scalar.dma_start(xT[:, 0:4, bass.ds(2 * QC, 2 * QC)],
                                xbT_r[:, 0:4, bass.ds(2 * QC, 2 * QC)])
            nc.sync.dma_start(xT[:, 4:8, bass.ds(2 * QC, 2 * QC)],
                              xbT_r[:, 4:8, bass.ds(2 * QC, 2 * QC)])
        elif kt == 4:  # chunks 4-7 in two batched issues
            nc.scalar.dma_start(xT[:, 0:4, bass.ds(4 * QC, 4 * QC)],
                                xbT_r[:, 0:4, bass.ds(4 * QC, 4 * QC)])
            nc.sync.dma_start(xT[:, 4:8, bass.ds(4 * QC, 4 * QC)],
                              xbT_r[:, 4:8, bass.ds(4 * QC, 4 * QC)])
        project(wk_sb, k_sb, kt)
        project(wv_sb, vT_sb, kt)
        if pending is not None:  # previous slot's finish overlaps these projs
            finish_slot(*pending)
            pending = None
        for j in range(4):  # PE-transpose V to token-major (keeps DMA in copy mode)
            bk = 4 * kt + j
            pstr = psTr.tile([P, P], BF16, name="tr")
            nc.tensor.transpose(pstr[:], vT_sb[:, bass.ts(bk, P)], ident[:])
            nc.vector.tensor_copy(v_sb[:, bk, 0:H], pstr[:])
        if kt % 2 == 0:
            project(wq_sb, q_sb, kt, dst_kt=kt // 2)  # QSLOT[g] == 2g == kt
        else:
            g = (kt - 1) // 2
            pending = (g, attention_slot(g))
    finish_slot(*pending)


_CACHED_NC = None


def _get_nc():
    global _CACHED_NC
    if _CACHED_NC is None:
        _CACHED_NC = _build_kernel()
    return _CACHED_NC


def _make_core_inputs(x, wqT, wkT, wvT, woT, core):
    b, role = core // 2, core % 2
    sigma = SIGMA[role]
    perm = np.concatenate([np.arange(QC) + c * QC for c in sigma])
    xbT = np.ascontiguousarray(x[perm, b, :].T.astype(ml_dtypes.bfloat16))

    # per-slot partner-block bias: partner true chunk before own -> keep (0),
    # after own -> fully masked (exp(s*SCALE - 50) == 0 in bf16)
    meta = np.zeros((P, NSLOT), np.float32)
    for g in range(NSLOT):
        if sigma[2 * g + 1] > sigma[2 * g]:
            meta[:, g] = MASK_BIAS
    return {"xbT": xbT, "wqT": wqT, "wkT": wkT, "wvT": wvT, "woT": woT,
            "meta": meta}


def _w_pch(w):
    """(H, D) weight -> [p, c, h] bf16 layout for a contiguous SBUF load."""
    return np.ascontiguousarray(
        np.asarray(w, np.float32).T.reshape(DC, P, H).transpose(1, 0, 2)
        .astype(ml_dtypes.bfloat16))


def kernel(x, Wq, Wk, Wv, Wo):
    x = np.asarray(x, dtype=np.float32)
    wqT = _w_pch(Wq)
    wkT = _w_pch(Wk)
    wvT = _w_pch(Wv)
    woT = np.ascontiguousarray(np.asarray(Wo, np.float32).T
                               .astype(ml_dtypes.bfloat16))

    nc = _get_nc()
    in_maps = [_make_core_inputs(x, wqT, wkT, wvT, woT, i) for i in range(8)]
    res = run_bass_kernel_spmd(nc, in_maps, list(range(8))).results

    out = np.empty((S, B, D), np.float32)
    for core in range(8):
        b, role = core // 2, core % 2
        sigma = SIGMA[role]
        co = np.asarray(res[core]["out"], dtype=np.float32)
        for g in range(NSLOT):
            c_g = sigma[QSLOT[g]]
            out[c_g * QC : (c_g + 1) * QC, b, :] = co[g * QC : (g + 1) * QC, :]
    return out
